# revision 1
# baseline (speedup 1.0000x reference)
"""Fused AttentionLocal kernel for 8 Trainium2 NeuronCores — width-FFT conv1.

conv1 (7x7 dilated-2) via mixed-domain convolution: rFFT-22 along width per
column parity (the dilated taps are 7 adjacent taps on each 16-wide parity
grid), height taps stay spatial. FFT-domain matmuls in fp16 (1 cyc/col on
PE, ~8x less quantization error than bf16).

BN folding identical to the direct-conv baseline; single fused AllReduce of
[sum(h); sum(h^2); G = h h^T].
"""

import contextlib

import numpy as np

import concourse.bass as bass
import concourse.tile as tile
from concourse import bacc, mybir
from concourse.masks import make_identity

F32 = mybir.dt.float32
F32R = mybir.dt.float32r
BF16 = mybir.dt.bfloat16
FP16 = mybir.dt.float16
AF = mybir.ActivationFunctionType
ALU = mybir.AluOpType
AX = mybir.AxisListType
EPS = 1e-5

N_CORES = 8
B_GLOBAL = 64
C = 256
HW = 1024
W2OUT = 1024
NF = 22     # circular FFT length for 16-wide parity grid, 7-tap conv
NFREQ = 12  # rfft bins 0..11

LOCAL_BN = False  # global-batch stats required: local stats fail the 2e-2 gate (6.6e-2)

KH_ORDER = [3, 0, 1, 2, 4, 5, 6]  # kh=3 (dy=0, full rows) first: start=True covers bank


def build_body(tc, aps, n_cores, b_loc, total_batch, dbg=False):
    nc = tc.nc
    P_TOT = float(total_batch * HW)
    x_ap = aps["x"]
    w1f_ap = aps["w1f"]
    w2t_ap = aps["w2t"]
    out_ap = aps["out"]

    ctx = contextlib.ExitStack()
    with ctx:
        persist = ctx.enter_context(tc.tile_pool(name="persist", bufs=1))
        dram = ctx.enter_context(tc.tile_pool(name="dram", bufs=1, space="DRAM"))

        # ---------------- prologue: constants + params ----------------
        ident16 = persist.tile([128, 128], FP16, tag="ident16", name="ident16")
        make_identity(nc, ident16[:])
        identb = persist.tile([128, 128], BF16, tag="identb", name="identb")
        make_identity(nc, identb[:])
        ones_f32 = persist.tile([128, 2], F32, tag="ones_f32", name="ones_f32")
        nc.gpsimd.memset(ones_f32[:], 1.0)
        ones_col = persist.tile([128, 1], F32R, tag="ones_col", name="ones_col")
        nc.vector.tensor_copy(ones_col[:], ones_f32[:, 0:1])

        dfwd = persist.tile([128, 192], FP16, tag="dfwd", name="dfwd")
        nc.sync.dma_start(dfwd[:], aps["dfwd"])
        dinv = persist.tile([96, 64], FP16, tag="dinv", name="dinv")

        def row_tile(name, src_1d, n):
            t = persist.tile([1, n], F32, tag=name, name=name)
            nc.sync.dma_start(t[:], src_1d.rearrange("(o n) -> o n", o=1))
            return t

        h_tiles = {}
        for b in range(b_loc):
            for oc in range(2):
                h_tiles[(b, oc)] = persist.tile([128, HW], BF16, tag=f"h{b}_{oc}", name=f"h{b}_{oc}")

        # xT tiles (pixel-major x, also attention operand): [128 pix, 256+2ones]
        xT = {}
        for b in range(b_loc):
            for j in range(8):
                xT[(b, j)] = persist.tile([128, 258], FP16, tag=f"xT{b}_{j}", name=f"xT{b}_{j}")

        s_acc = [persist.tile([128, b_loc], F32, tag=f"sacc{oc}", name=f"sacc{oc}") for oc in range(2)]
        q_acc = [persist.tile([128, b_loc], F32, tag=f"qacc{oc}", name=f"qacc{oc}") for oc in range(2)]
        s_col = [persist.tile([128, 1], F32, tag=f"scol{oc}", name=f"scol{oc}") for oc in range(2)]
        q_col = [persist.tile([128, 1], F32, tag=f"qcol{oc}", name=f"qcol{oc}") for oc in range(2)]
        bn1pp = [persist.tile([128, 3], F32R, tag=f"bn1pp{oc}", name=f"bn1pp{oc}") for oc in range(2)]
        bn2pp = persist.tile([128, 16], F32, tag="bn2pp", name="bn2pp")

        sq_in = dram.tile([2, C], F32, tag="sq_in", name="sq_in")
        sq_out = dram.tile([2, C], F32, tag="sq_out", name="sq_out")
        g_in = dram.tile([C, C], F32, tag="g_in", name="g_in")
        g_out = dram.tile([C, C], F32, tag="g_out", name="g_out")
        bn1_bounce = dram.tile([2, C], F32, tag="bn1_bounce", name="bn1_bounce")
        bn2_bounce = dram.tile([3, W2OUT], F32, tag="bn2_bounce", name="bn2_bounce")

        # hhat[b][oc]: [co128, (rows32, par2, fx12, ri2)=1536] fp16, per img
        with tc.tile_pool(name="hhp", bufs=1) as hhp:
            hhat = {}
            for b in range(b_loc):
                for oc in range(2):
                    hhat[(b, oc)] = hhp.tile([128, 32 * 2 * NFREQ * 2], FP16,
                                             tag=f"hh{b}_{oc}", name=f"hh{b}_{oc}")

            with tc.tile_pool(name="xhp", bufs=1) as xhp:
                # xhat[cc]: [cin, (v2(re,im), fx12, par2, img8, rows32)] fp16
                XFREE = 2 * NFREQ * 2 * 32 * b_loc
                xhat = [xhp.tile([128, XFREE], FP16, tag=f"xhat{cc}", name=f"xhat{cc}")
                        for cc in range(2)]
                xv = [xhat[cc][:].rearrange("p (v f q i r) -> p v f q i r",
                                            v=2, f=NFREQ, q=2, i=b_loc, r=32)
                      for cc in range(2)]
                xvm = [xhat[cc][:].rearrange("p (g q i r) -> p g q i r",
                                             g=2 * NFREQ, q=2, i=b_loc)
                       for cc in range(2)]

                # ------------ phase F: x transposes + forward width-DFT --------
                # pipelined by one image: img b-1's DFT matmuls are emitted
                # after img b's transposes, so the tensor queue never stalls
                # on the DVE xT copies.
                with tc.tile_pool(name="fin", bufs=2) as fin, \
                     tc.tile_pool(name="ftp", bufs=3, space="PSUM") as ftp, \
                     tc.tile_pool(name="fps", bufs=5, space="PSUM") as fps:

                    def emit_fwd(b):
                        for j in range(8):
                            xt_ = xT[(b, j)]
                            for cc in range(2):
                                fo = fps.tile([128, 192], F32, tag="fo", name="fo")
                                nc.tensor.matmul(fo[:], xt_[:, cc * 128:(cc + 1) * 128],
                                                 dfwd[:])
                                dst = xvm[cc][:, :, :, b, j * 4:(j + 1) * 4]
                                src = fo[:].rearrange("p (g q r) -> p g q r",
                                                      g=2 * NFREQ, q=2)
                                if cc == 0:
                                    nc.vector.tensor_copy(dst, src)
                                else:
                                    nc.scalar.copy(dst, src)

                    for b in range(b_loc):
                        x16 = []
                        for cc in range(2):
                            xin = fin.tile([128, HW], F32, tag=f"xin{cc}", name=f"xin{cc}")
                            nc.sync.dma_start(xin[:], x_ap[b, cc * 128:(cc + 1) * 128, :])
                            x1 = fin.tile([128, HW], FP16, tag=f"x16_{cc}", name=f"x16_{cc}")
                            nc.scalar.copy(x1[:], xin[:])
                            x16.append(x1)
                        for j in range(8):
                            xt_ = xT[(b, j)]
                            for cc in range(2):
                                tp = ftp.tile([128, 128], FP16, tag="tp", name="tp")
                                nc.tensor.matmul(tp[:], x16[cc][:, j * 128:(j + 1) * 128],
                                                 ident16[:], is_transpose=True)
                                nc.vector.tensor_copy(xt_[:, cc * 128:(cc + 1) * 128], tp[:])
                            nc.vector.tensor_copy(xt_[:, 256:258], ones_f32[:])
                        if b > 0:
                            emit_fwd(b - 1)
                    emit_fwd(b_loc - 1)

                # params not needed until later (off the startup critical path)
                nc.sync.dma_start(dinv[:], aps["dinv"])
                g1row = row_tile("g1row", aps["bn1g"], C)
                b1row = row_tile("b1row", aps["bn1b"], C)
                cb2row = row_tile("cb2row", aps["cb2"], W2OUT)
                w2t_t = []
                for cc in range(2):
                    t = persist.tile([128, W2OUT], F32R, tag=f"w2t{cc}", name=f"w2t{cc}")
                    nc.sync.dma_start(t[:], w2t_ap[cc * 128:(cc + 1) * 128, :].bitcast(F32R))
                    w2t_t.append(t)

                # ------------ phase P: pointwise (freq-domain conv) ------------
                with tc.tile_pool(name="wtp", bufs=2) as wtp, \
                     tc.tile_pool(name="pps", bufs=3, space="PSUM") as pps:
                    for fx in range(NFREQ):
                        for co in range(2):
                            # weights: [cin, (kh7, wv3, cc2) x 128co]
                            wt = wtp.tile([128, 42 * 128], FP16, tag="wt", name="wt")
                            nc.sync.dma_start(
                                wt[:].rearrange("p (g o) -> p g o", g=42),
                                w1f_ap[fx, :, :, co * 128:(co + 1) * 128]
                                .rearrange("g p o -> p g o"))
                            # fx 0 and 11: Wim == 0 and xhat_im == 0 => only
                            # the Wre*xre product contributes; him left stale
                            # (killed by zero rows of dinv).
                            realonly = fx in (0, NFREQ - 1)
                            hre = pps.tile([128, 512], F32, tag="hre", name="hre")
                            him = pps.tile([128, 512], F32, tag="him", name="him")
                            hrev = hre[:].rearrange("p (q i r) -> p q i r", q=2, i=b_loc)
                            himv = him[:].rearrange("p (q i r) -> p q i r", q=2, i=b_loc)
                            if realonly:
                                # him := Wre * xhat_im == exact zeros (keeps the
                                # bank initialized; dinv rows for these (fx,ri=1)
                                # are zero anyway)
                                nc.tensor.matmul(
                                    him[:], wt[:, (3 * 6) * 128:(3 * 6 + 1) * 128],
                                    xv[0][:, 1, fx, :, :, :],
                                    start=True, stop=True, skip_group_check=True)
                            first = True
                            for kh in KH_ORDER:
                                dy = 2 * kh - 6
                                r0 = max(0, -dy)
                                r1 = min(32, 32 - dy)
                                od = hrev[:, :, :, r0:r1]
                                oi = himv[:, :, :, r0:r1]
                                last = (kh == KH_ORDER[-1])
                                for cc in range(2):
                                    xre = xv[cc][:, 0, fx, :, :, r0 + dy:r1 + dy]
                                    xim = xv[cc][:, 1, fx, :, :, r0 + dy:r1 + dy]
                                    lcc = (last and cc == 1)

                                    def wslice(wv):
                                        g = kh * 6 + wv * 2 + cc
                                        return wt[:, g * 128:(g + 1) * 128]

                                    # Wre: hre += Wre*xre ; him += Wre*xim
                                    nc.tensor.matmul(od, wslice(0), xre,
                                                     start=first, stop=lcc and realonly,
                                                     skip_group_check=True)
                                    if realonly:
                                        first = False
                                        continue
                                    nc.tensor.matmul(oi, wslice(0), xim,
                                                     start=first, stop=False,
                                                     skip_group_check=True)
                                    first = False
                                    # Wim: him += Wim*xre ; -Wim: hre += -Wim*xim
                                    nc.tensor.matmul(oi, wslice(1), xre,
                                                     start=False, stop=lcc,
                                                     skip_group_check=True)
                                    nc.tensor.matmul(od, wslice(2), xim,
                                                     start=False, stop=lcc,
                                                     skip_group_check=True)
                            # drain psum -> hhat per img; hhat layout (r q f i)
                            for b in range(b_loc):
                                for ri, ps in ((0, hrev), (1, himv)):
                                    dst = hhat[(b, co)][:].rearrange(
                                        "p (r q f i) -> p q r f i",
                                        r=32, q=2, f=NFREQ)[:, :, :, fx, ri]
                                    nc.vector.tensor_copy(dst, ps[:, :, b, :])

                if dbg:
                    for cc in range(2):
                        nc.sync.dma_start(aps["xhat_d"][cc], xhat[cc][:])

            # ------------ phase I: inverse DFT, then stats + gram --------------
            with tc.tile_pool(name="itp", bufs=4, space="PSUM") as itp, \
                 tc.tile_pool(name="ips", bufs=4, space="PSUM") as ips, \
                 tc.tile_pool(name="ht", bufs=4) as htpool, \
                 tc.tile_pool(name="gscr", bufs=2) as gscr:
                for b in range(b_loc):
                    for oc in range(2):
                        hh = hhat[(b, oc)]
                        for rp in range(16):
                            tp = itp.tile([96, 128], FP16, tag="itp", name="itp")
                            nc.tensor.matmul(tp[:], hh[:, rp * 96:(rp + 1) * 96],
                                             ident16[:], is_transpose=True)
                            hT96 = htpool.tile([96, 128], FP16, tag="hT96", name="hT96")
                            nc.vector.tensor_copy(hT96[:], tp[:])
                            io = ips.tile([128, 64], F32, tag="io", name="io")
                            nc.tensor.matmul(io[:], hT96[:], dinv[:])
                            nc.scalar.copy(h_tiles[(b, oc)][:, rp * 64:(rp + 1) * 64],
                                           io[:])
                    # per-image stats (vector/scalar engines, off tensor path)
                    for oc in range(2):
                        nc.vector.reduce_sum(
                            s_acc[oc][:, b:b + 1], h_tiles[(b, oc)][:], axis=AX.X)
                        scr = gscr.tile([128, HW], F32, tag="ttr", name="ttr")
                        nc.scalar.activation(
                            scr[:], h_tiles[(b, oc)][:], AF.Square,
                            accum_out=q_acc[oc][:, b:b + 1])

                # sq collective fires before gram: phase 3's BN1 chain only
                # needs it, and then overlaps the G collective latency
                for oc in range(2):
                    nc.vector.reduce_sum(s_col[oc][:], s_acc[oc][:], axis=AX.X)
                    nc.vector.reduce_sum(q_col[oc][:], q_acc[oc][:], axis=AX.X)
                    nc.sync.dma_start(
                        sq_in[0:1, oc * 128:(oc + 1) * 128].rearrange("o p -> p o"),
                        s_col[oc][:])
                    nc.sync.dma_start(
                        sq_in[1:2, oc * 128:(oc + 1) * 128].rearrange("o p -> p o"),
                        q_col[oc][:])
                nc.gpsimd.collective_compute(
                    "AllReduce", ALU.add,
                    replica_groups=[list(range(n_cores))],
                    ins=[sq_in.opt()],
                    outs=[sq_out.opt()])

            if dbg:
                for b in range(b_loc):
                    for oc in range(2):
                        nc.sync.dma_start(aps["hh_d"][b * 2 + oc], hhat[(b, oc)][:])
                        nc.sync.dma_start(aps["h_d"][b * 2 + oc], h_tiles[(b, oc)][:])

            with tc.tile_pool(name="gtp", bufs=3, space="PSUM") as gtp, \
                 tc.tile_pool(name="gscr", bufs=2) as gscr, \
                 tc.tile_pool(name="hTp", bufs=3) as hTp, \
                 tc.tile_pool(name="gps", bufs=1, space="PSUM") as gps_pool:
                gps = [gps_pool.tile([128, C], F32, tag=f"gps{oc}", name=f"gps{oc}")
                       for oc in range(2)]
                for b in range(b_loc):
                    for j in range(8):
                        hT = hTp.tile([128, C], BF16, tag="hT", name="hT")
                        for oc in range(2):
                            tpb = gtp.tile([128, 128], BF16, tag="tpb", name="tpb")
                            nc.tensor.matmul(
                                tpb[:], h_tiles[(b, oc)][:, j * 128:(j + 1) * 128],
                                identb[:], is_transpose=True)
                            nc.vector.tensor_copy(hT[:, oc * 128:(oc + 1) * 128], tpb[:])
                        for oc in range(2):
                            first = (b == 0 and j == 0)
                            last = (b == b_loc - 1 and j == 7)
                            nc.tensor.matmul(
                                gps[oc][:], hT[:, oc * 128:(oc + 1) * 128], hT[:],
                                start=first, stop=last, skip_group_check=True)

                for oc in range(2):
                    gsb = gscr.tile([128, C], F32, tag=f"gsb{oc}", name=f"gsb{oc}")
                    nc.vector.tensor_copy(gsb[:], gps[oc][:])
                    nc.sync.dma_start(g_in[oc * 128:(oc + 1) * 128, :], gsb[:])

                nc.gpsimd.collective_compute(
                    "AllReduce", ALU.add,
                    replica_groups=[list(range(n_cores))],
                    ins=[g_in.opt()],
                    outs=[g_out.opt()])

        # ---------------- phase 3: BN constants from global stats ----------
        with tc.tile_pool(name="rows", bufs=1) as rows, \
             tc.tile_pool(name="prodp", bufs=2) as prodp, \
             tc.tile_pool(name="rowps", bufs=2, space="PSUM") as rowps, \
             tc.tile_pool(name="m1ps", bufs=2, space="PSUM") as m1ps:

            def rt(name, n=C):
                return rows.tile([1, n], F32, tag=name, name=name)

            s_row = rt("s_row")
            nc.sync.dma_start(s_row[:], sq_out[0:1, :])
            q_row = rt("q_row")
            nc.sync.dma_start(q_row[:], sq_out[1:2, :])

            meanh = rt("meanh")
            nc.vector.tensor_scalar_mul(meanh[:], s_row[:], 1.0 / P_TOT)
            msq = rt("msq")
            nc.vector.tensor_mul(msq[:], meanh[:], meanh[:])
            var1 = rt("var1")
            nc.vector.tensor_scalar_mul(var1[:], q_row[:], 1.0 / P_TOT)
            nc.vector.tensor_sub(var1[:], var1[:], msq[:])
            nc.vector.tensor_scalar_add(var1[:], var1[:], EPS)
            rec1 = rt("rec1")
            nc.vector.reciprocal(rec1[:], var1[:])
            rsq1 = rt("rsq1")
            nc.scalar.activation(rsq1[:], rec1[:], AF.Sqrt)
            a1row = rt("a1row")
            nc.vector.tensor_mul(a1row[:], rsq1[:], g1row[:])
            tmp1 = rt("tmp1")
            nc.vector.tensor_mul(tmp1[:], a1row[:], meanh[:])
            c1srow = rt("c1srow")
            nc.vector.tensor_sub(c1srow[:], b1row[:], tmp1[:])

            nc.sync.dma_start(bn1_bounce[0:1, :], a1row[:])
            nc.sync.dma_start(bn1_bounce[1:2, :], c1srow[:])
            for oc in range(2):
                nc.sync.dma_start(
                    bn1pp[oc][:, 0:2],
                    bn1_bounce[:, oc * 128:(oc + 1) * 128]
                    .rearrange("r p -> p r").bitcast(F32R))
                nc.sync.dma_start(
                    bn1pp[oc][:, 2:3],
                    sq_out[0:1, oc * 128:(oc + 1) * 128]
                    .rearrange("o p -> p o").bitcast(F32R))

            # cst[n] = sum_c W2[n,c] * c1s[c] + conv2_b[n]  (unscaled W2)
            cstrow = rt("cstrow", W2OUT)
            for nh in range(2):
                cp_ = rowps.tile([1, 512], F32, tag="rowps", name="rowps")
                for oc in range(2):
                    nc.tensor.matmul(
                        cp_[:], bn1pp[oc][:, 1:2], w2t_t[oc][:, nh * 512:(nh + 1) * 512],
                        start=(oc == 0), stop=(oc == 1), skip_group_check=True)
                nc.vector.tensor_add(
                    cstrow[:, nh * 512:(nh + 1) * 512], cp_[0:1, :],
                    cb2row[:, nh * 512:(nh + 1) * 512])

            # scale W2T in place by a1 (per input channel)
            for oc in range(2):
                nc.vector.tensor_scalar_mul(
                    w2t_t[oc][:], w2t_t[oc][:], bn1pp[oc][:, 0:1].bitcast(F32))

            # r1[n] = sum_c W2'[n,c] * s[c]  (scaled W2, unscaled s)
            r1row = rt("r1row", W2OUT)
            for nh in range(2):
                rp_ = rowps.tile([1, 512], F32, tag="rowps", name="rowps")
                for oc in range(2):
                    nc.tensor.matmul(
                        rp_[:], bn1pp[oc][:, 2:3], w2t_t[oc][:, nh * 512:(nh + 1) * 512],
                        start=(oc == 0), stop=(oc == 1), skip_group_check=True)
                nc.vector.tensor_copy(r1row[:, nh * 512:(nh + 1) * 512], rp_[0:1, :])

            nc.sync.dma_start(bn2_bounce[0:1, :], cstrow[:])
            nc.sync.dma_start(bn2_bounce[1:2, :], r1row[:])
            g2pp = rows.tile([128, 8], F32, tag="g2pp", name="g2pp")
            nc.sync.dma_start(g2pp[:], aps["bn2g"].rearrange("(k p) -> p k", p=128))
            b2pp = rows.tile([128, 8], F32, tag="b2pp", name="b2pp")
            nc.sync.dma_start(b2pp[:], aps["bn2b"].rearrange("(k p) -> p k", p=128))

            # M1 = G @ W2'^T ; e[n] = sum_c W2'[n,c] * M1[c,n]
            g_glob = []
            for oc in range(2):
                gg = rows.tile([128, C], F32R, tag=f"gglob{oc}", name=f"gglob{oc}")
                nc.sync.dma_start(
                    gg[:], g_out[oc * 128:(oc + 1) * 128, :].bitcast(F32R))
                g_glob.append(gg)
            M1 = [rows.tile([128, W2OUT], F32R, tag=f"M1_{oc}", name=f"M1_{oc}") for oc in range(2)]
            for occ in range(2):
                for nh in range(2):
                    mp = m1ps.tile([128, 512], F32, tag="m1ps", name="m1ps")
                    for dd in range(2):
                        nc.tensor.matmul(
                            mp[:], g_glob[dd][:, occ * 128:(occ + 1) * 128],
                            w2t_t[dd][:, nh * 512:(nh + 1) * 512],
                            start=(dd == 0), stop=(dd == 1), skip_group_check=True)
                    nc.vector.tensor_copy(M1[occ][:, nh * 512:(nh + 1) * 512], mp[:])
            erow = rt("erow", W2OUT)
            prods = []
            for oc in range(2):
                pr = prodp.tile([128, W2OUT], F32R, tag="prod", name="prod")
                nc.vector.tensor_mul(pr[:], w2t_t[oc][:].bitcast(F32), M1[oc][:].bitcast(F32))
                prods.append(pr)
            for nh in range(2):
                ep_ = rowps.tile([1, 512], F32, tag="rowps", name="rowps")
                for oc in range(2):
                    nc.tensor.matmul(
                        ep_[:], ones_col[:], prods[oc][:, nh * 512:(nh + 1) * 512],
                        start=(oc == 0), stop=(oc == 1), skip_group_check=True)
                nc.vector.tensor_copy(erow[:, nh * 512:(nh + 1) * 512], ep_[0:1, :])

            # BN2 constants in per-partition [128, 8] layout
            nc.sync.dma_start(bn2_bounce[2:3, :], erow[:])
            cre = rows.tile([128, 24], F32, tag="cre", name="cre")
            nc.sync.dma_start(
                cre[:], bn2_bounce[:].rearrange("w (k p) -> p (w k)", p=128))
            cstp, r1p, ep = cre[:, 0:8], cre[:, 8:16], cre[:, 16:24]

            def pp(name):
                return rows.tile([128, 8], F32, tag=name, name=name)

            mkp = pp("mkp")
            nc.vector.tensor_scalar_mul(mkp[:], r1p, 1.0 / P_TOT)
            nc.vector.tensor_add(mkp[:], mkp[:], cstp)
            t1p = pp("t1p")
            nc.vector.tensor_mul(t1p[:], cstp, r1p)
            nc.vector.tensor_scalar_mul(t1p[:], t1p[:], 2.0 / P_TOT)
            t2p = pp("t2p")
            nc.vector.tensor_mul(t2p[:], cstp, cstp)
            ek2p = pp("ek2p")
            nc.vector.tensor_scalar_mul(ek2p[:], ep, 1.0 / P_TOT)
            nc.vector.tensor_add(ek2p[:], ek2p[:], t1p[:])
            nc.vector.tensor_add(ek2p[:], ek2p[:], t2p[:])
            nc.vector.tensor_mul(t1p[:], mkp[:], mkp[:])
            nc.vector.tensor_sub(ek2p[:], ek2p[:], t1p[:])
            nc.vector.tensor_scalar_add(ek2p[:], ek2p[:], EPS)
            nc.vector.reciprocal(t2p[:], ek2p[:])
            nc.scalar.activation(t1p[:], t2p[:], AF.Sqrt)
            nc.vector.tensor_mul(bn2pp[:, 0:8], t1p[:], g2pp[:])
            nc.vector.tensor_scalar_mul(t2p[:], r1p, 1.0 / P_TOT)
            nc.vector.tensor_mul(t2p[:], bn2pp[:, 0:8], t2p[:])
            nc.vector.tensor_sub(bn2pp[:, 8:16], b2pp[:], t2p[:])

        # ---------------- phase 4: conv2 + exp + attention ------------------
        with tc.tile_pool(name="w2bp", bufs=1) as w2bp, \
             tc.tile_pool(name="kexp", bufs=12) as kexp_pool, \
             tc.tile_pool(name="outp", bufs=4) as outp, \
             tc.tile_pool(name="recp", bufs=4) as recp, \
             tc.tile_pool(name="c2ps", bufs=3, space="PSUM") as c2ps, \
             tc.tile_pool(name="aps", bufs=3, space="PSUM") as aps_pool:
            # bf16 copy of the scaled conv2 weights
            w2tb = []
            for cc in range(2):
                wb = w2bp.tile([128, W2OUT], BF16, tag=f"w2tb{cc}", name=f"w2tb{cc}")
                nc.vector.tensor_copy(wb[:], w2t_t[cc][:].bitcast(F32))
                w2tb.append(wb)
            for b in range(b_loc):
                ke = []
                for j in range(8):
                    for hf in range(2):
                        cp_ = c2ps.tile([128, 512], F32, tag="c2ps", name="c2ps", bufs=3)
                        for cc in range(2):
                            nc.tensor.matmul(
                                cp_[:], w2tb[cc][:, j * 128:(j + 1) * 128],
                                h_tiles[(b, cc)][:, hf * 512:(hf + 1) * 512],
                                start=(cc == 0), stop=(cc == 1), skip_group_check=True)
                        ket = kexp_pool.tile([128, 512], FP16, tag=f"ke{hf}", name=f"ke{hf}", bufs=10)
                        nc.scalar.activation(
                            ket[:], cp_[:], AF.Exp,
                            bias=bn2pp[:, 8 + j:9 + j], scale=bn2pp[:, j:j + 1])
                        ke.append(ket)
                for pc in range(8):
                    ap_ = aps_pool.tile([128, 258], F32, tag="aps", name="aps")
                    hf, pcl = pc // 4, pc % 4
                    for j in range(8):
                        nc.tensor.matmul(
                            ap_[:], ke[j * 2 + hf][:, pcl * 128:(pcl + 1) * 128],
                            xT[(b, j)][:],
                            start=(j == 0), stop=(j == 7), skip_group_check=True)
                    rec = recp.tile([128, 1], F32, tag="rec", name="rec")
                    nc.vector.reciprocal(rec[:], ap_[:, 256:257])
                    osb = outp.tile([128, C], F32, tag="osb", name="osb")
                    nc.vector.tensor_scalar_mul(osb[:], ap_[:, 0:256], rec[:])
                    r0 = pc * 128
                    nc.sync.dma_start(out_ap[b, r0:r0 + 128, :], osb[:])


def build(n_cores=N_CORES, b_loc=B_GLOBAL // N_CORES, total_batch=B_GLOBAL, dbg=False):
    nc = bacc.Bacc("TRN2", target_bir_lowering=False, debug=False, num_devices=n_cores)
    aps = {
        "x": nc.dram_tensor("x", [b_loc, C, HW], F32, kind="ExternalInput").ap(),
        # w1f: [fx, (kh7 wv3 cc2)=42, cin128, co256] fp16; wv = (re, im, -im)
        "w1f": nc.dram_tensor("w1f", [NFREQ, 42, 128, C], FP16, kind="ExternalInput").ap(),
        "dfwd": nc.dram_tensor("dfwd", [128, 192], FP16, kind="ExternalInput").ap(),
        "dinv": nc.dram_tensor("dinv", [96, 64], FP16, kind="ExternalInput").ap(),
        "w2t": nc.dram_tensor("w2t", [C, W2OUT], F32, kind="ExternalInput").ap(),
        "bn1g": nc.dram_tensor("bn1g", [C], F32, kind="ExternalInput").ap(),
        "bn1b": nc.dram_tensor("bn1b", [C], F32, kind="ExternalInput").ap(),
        "bn2g": nc.dram_tensor("bn2g", [W2OUT], F32, kind="ExternalInput").ap(),
        "bn2b": nc.dram_tensor("bn2b", [W2OUT], F32, kind="ExternalInput").ap(),
        "cb2": nc.dram_tensor("cb2", [W2OUT], F32, kind="ExternalInput").ap(),
        "out": nc.dram_tensor("out", [b_loc, HW, C], F32, kind="ExternalOutput").ap(),
    }
    if dbg:
        XFREE = 2 * NFREQ * 2 * 32 * b_loc
        aps["xhat_d"] = nc.dram_tensor("xhat_d", [2, 128, XFREE], FP16, kind="ExternalOutput").ap()
        aps["hh_d"] = nc.dram_tensor("hh_d", [2 * b_loc, 128, 32 * 2 * NFREQ * 2], FP16, kind="ExternalOutput").ap()
        aps["h_d"] = nc.dram_tensor("h_d", [2 * b_loc, 128, HW], BF16, kind="ExternalOutput").ap()
    with tile.TileContext(nc) as tc:
        build_body(tc, aps, n_cores, b_loc, total_batch, dbg=dbg)
    nc.compile()
    return nc


_CACHE = {}


def _host_fft_consts():
    j = np.arange(16)
    fx = np.arange(NFREQ)
    ang = 2 * np.pi * np.outer(j + 3, fx) / NF          # [16, 12]
    Dre = np.cos(ang)
    Dim = -np.sin(ang)
    # dfwd [128 = (rl4 x 32cols), 192 = (v2, fx12, par2, rl4)]
    dfwd = np.zeros((128, 2, NFREQ, 2, 4), np.float32)
    for q in range(128):
        rl, c = q // 32, q % 32
        par, jj = c % 2, c // 2
        dfwd[q, 0, :, par, rl] = Dre[jj]
        dfwd[q, 1, :, par, rl] = Dim[jj]
    dfwd = dfwd.reshape(128, 192).astype(np.float16)
    # dinv [96 = (rl2, par2, fx12, ri2), 64 = (rl2, 32cols)]
    angi = 2 * np.pi * np.outer(fx, j + 3) / NF          # [12, 16]
    sc = np.full((NFREQ, 1), 2.0)
    sc[0, 0] = 1.0
    sc[NFREQ - 1, 0] = 1.0
    Ire = np.cos(angi) * sc / NF
    Iim = -np.sin(angi) * sc / NF
    dinv = np.zeros((2, 2, NFREQ, 2, 2, 32), np.float32)
    for c in range(32):
        par, jj = c % 2, c // 2
        for rl in range(2):
            dinv[rl, par, :, 0, rl, c] = Ire[:, jj]
            dinv[rl, par, :, 1, rl, c] = Iim[:, jj]
    dinv = dinv.reshape(96, 64).astype(np.float16)
    return dfwd, dinv


def _prep_in_maps(inputs, n_cores, b_loc):
    w1 = np.asarray(inputs["conv1_w"], np.float32)       # [co, cin, kh, kw]
    t = np.arange(7) - 3
    fx = np.arange(NFREQ)
    angw = 2 * np.pi * np.outer(t, fx) / NF
    Wre = np.cos(angw)                                    # e^{+i 2pi fx t/22}
    Wim = np.sin(angw)
    whre = np.einsum('oikt,tf->fkio', w1, Wre, optimize=True)
    whim = np.einsum('oikt,tf->fkio', w1, Wim, optimize=True)
    # w1f [fx, kh, wv3, cin256, co] -> [fx, 42, 128, co]
    w1f = np.stack([whre, whim, -whim], axis=2)
    w1f = np.ascontiguousarray(w1f.reshape(NFREQ, 42, 128, C)).astype(np.float16)
    dfwd, dinv = _host_fft_consts()

    w2t = np.ascontiguousarray(np.asarray(inputs["conv2_w"], np.float32)[:, :, 0, 0].T)
    shared = {
        "w1f": w1f,
        "dfwd": dfwd,
        "dinv": dinv,
        "w2t": w2t,
        "bn1g": np.asarray(inputs["bn1_g"], np.float32),
        "bn1b": np.asarray(inputs["bn1_b"], np.float32),
        "bn2g": np.asarray(inputs["bn2_g"], np.float32),
        "bn2b": np.asarray(inputs["bn2_b"], np.float32),
        "cb2": np.asarray(inputs["conv2_b"], np.float32),
    }
    x = np.asarray(inputs["x"], np.float32).reshape(-1, C, HW)
    in_maps = []
    for i in range(n_cores):
        m = dict(shared)
        m["x"] = np.ascontiguousarray(x[i * b_loc:(i + 1) * b_loc])
        in_maps.append(m)
    return in_maps


def kernel(**inputs):
    from concourse import bass_utils
    b_loc = B_GLOBAL // N_CORES
    if "nc" not in _CACHE:
        _CACHE["nc"] = build(N_CORES, b_loc, b_loc if LOCAL_BN else B_GLOBAL)
    nc = _CACHE["nc"]
    in_maps = _prep_in_maps(inputs, N_CORES, b_loc)
    res = bass_utils.run_bass_kernel_spmd(nc, in_maps, core_ids=list(range(N_CORES)))
    y = np.concatenate([res.results[i]["out"] for i in range(N_CORES)], axis=0)
    return np.ascontiguousarray(y).reshape(B_GLOBAL, C, 32, 32)

